# revision 7
# baseline (speedup 1.0000x reference)
"""Trainium2 Bass kernel for nn_CachedVideoAttention (B=2, S=2048, D=512, H=8).

Sharding: 8 cores = 2 batches x 4 head-pairs. Core c handles batch c//4 and
heads {2*(c%4), 2*(c%4)+1}. Each core projects q/k/v for its heads, runs
causal (or general-mask) attention in a transposed-score layout with a
max-free softmax, applies its slice of the output projection, and a
ReduceScatter over each batch's 4 cores yields each core's 512-row shard of
the final output. The host only packs/slices numpy inputs and concatenates
the 8 output shards.
"""

import numpy as np

import concourse.bass as bass
import concourse.mybir as mybir
import concourse.tile as tile
from concourse import bacc
from concourse.bass_utils import run_bass_kernel_spmd

B, S, D, H, DH = 2, 2048, 512, 8, 64
EPS = 1e-6
N_CORES = 8
HP = 2                       # heads per core
TOK = 128                    # token chunk (partition tile)
NT = S // TOK                # 16 token chunks
QC = 512                     # query chunk for attention
NQ = S // QC                 # 4 query chunks
NKB = S // TOK               # 16 key blocks
WCOLS = 3 * HP * DH + HP     # packed projection output cols: 384 qkv + 2 ones
VOFF = 2 * HP * DH           # offset of v/ones block in packed cols (256)
F32 = mybir.dt.float32
F32R = mybir.dt.float32r
U8 = mybir.dt.uint8
AF = mybir.ActivationFunctionType
ALU = mybir.AluOpType

_cache = {}


def _register_const(nc, val, dtype=F32):
    key = (dtype, float(val))
    if key in nc.const_aps.aps:
        return
    t = nc.alloc_sbuf_tensor(f"const-{dtype.name}-{val}", [128, 1], dtype)
    nc.gpsimd.memset(t.ap(), val)
    nc.const_aps.aps[key] = t.ap()


def _kb_count(qc, mask_mode):
    # number of key blocks for query chunk qc
    if mask_mode == "causal":
        return 4 * qc + 4
    return NKB


def _build(mask_mode, has_bo, expC):
    nc = bacc.Bacc("TRN2", target_bir_lowering=False, debug=False,
                   enable_asserts=True, num_devices=N_CORES)

    i_xT = nc.dram_tensor("xT", [D, S], F32R, kind="ExternalInput")
    i_Wp = nc.dram_tensor("Wp", [D, WCOLS], F32R, kind="ExternalInput")
    i_Wb = nc.dram_tensor("Wb", [1, WCOLS], F32R, kind="ExternalInput")
    i_Wo = nc.dram_tensor("WoP", [HP * DH, D], F32R, kind="ExternalInput")
    i_wqk = nc.dram_tensor("wqk", [HP * DH, 1], F32, kind="ExternalInput")
    i_ident = nc.dram_tensor("ident", [128, 128], F32R, kind="ExternalInput")
    i_ones = nc.dram_tensor("onesr", [1, S], F32R, kind="ExternalInput")
    if mask_mode == "causal":
        i_strip = nc.dram_tensor("strip", [TOK, TOK + QC + 256], F32,
                                 kind="ExternalInput")
    elif mask_mode == "general":
        i_maskT = nc.dram_tensor("maskT", [S, S], U8, kind="ExternalInput")
    if has_bo:
        i_bo = nc.dram_tensor("boB", [TOK, D], F32, kind="ExternalInput")
    o_y = nc.dram_tensor("y", [S // 4, D], F32, kind="ExternalOutput")

    d_part = nc.dram_tensor("opart", [S, D], F32)
    d_red = nc.dram_tensor("ored", [S // 4, D], F32)

    _register_const(nc, float(-expC))
    _register_const(nc, float(EPS))
    _register_const(nc, float(1.0 / np.sqrt(DH)))

    with tile.TileContext(nc) as tc:
        with tc.tile_pool(name="res", bufs=1) as rp, \
             tc.tile_pool(name="work", bufs=3) as wp, \
             tc.tile_pool(name="ps_proj", bufs=2, space="PSUM") as pproj, \
             tc.tile_pool(name="ps_tr", bufs=2, space="PSUM") as ptr, \
             tc.tile_pool(name="ps_s", bufs=2, space="PSUM") as pscore, \
             tc.tile_pool(name="ps_o", bufs=2, space="PSUM") as pout:

            # ---- resident loads ----
            ident = rp.tile([128, 128], F32R, tag="ident")
            nc.sync.dma_start(ident[:, :], i_ident[:, :])
            tWp = []
            for c in range(4):
                t = rp.tile([128, WCOLS], F32R, tag=f"Wp{c}", name=f"Wp{c}")
                nc.sync.dma_start(t[:, :], i_Wp[c * 128:(c + 1) * 128, :])
                tWp.append(t)
            tWb = rp.tile([1, WCOLS], F32R, tag="Wb")
            nc.sync.dma_start(tWb[:, :], i_Wb[:, :])
            tWo = rp.tile([HP * DH, D], F32R, tag="Wo")
            nc.sync.dma_start(tWo[:, :], i_Wo[:, :])
            twqk = rp.tile([HP * DH, 1], F32, tag="wqk")
            nc.sync.dma_start(twqk[:, :], i_wqk[:, :])
            if mask_mode == "causal":
                tstrip = rp.tile([TOK, TOK + QC + 256], F32, tag="strip")
                nc.sync.dma_start(tstrip[:, :], i_strip[:, :])
            if has_bo:
                tbo = rp.tile([TOK, D], F32, tag="bo")
                nc.sync.dma_start(tbo[:, :], i_bo[:, :])
            txT = []
            for c in range(4):
                t = rp.tile([128, S], F32R, tag=f"xT{c}", name=f"xT{c}")
                nc.sync.dma_start(t[:, :], i_xT[c * 128:(c + 1) * 128, :])
                txT.append(t)
            ones_row = rp.tile([1, S], F32R, tag="ones")
            nc.sync.dma_start(ones_row[:, :], i_ones[:, :])

            # ---- qkv projection (natural layout) + rmsnorm + transpose ----
            qT = rp.tile([2 * DH, S], F32R, tag="qT")
            kT = rp.tile([2 * DH, S], F32R, tag="kT")
            attnT = rp.tile([2 * DH, S], F32R, tag="attnT")
            vaug = []
            for t in range(NT):
                vaug.append(rp.tile([TOK, HP * (DH + 1)], F32R, tag=f"vaug{t}", name=f"vaug{t}"))

            for t in range(NT):
                ps = pproj.tile([TOK, WCOLS], F32, tag="ps")
                for c in range(4):
                    nc.tensor.matmul(
                        ps[:, :], txT[c][:, t * TOK:(t + 1) * TOK], tWp[c][:, :],
                        start=(c == 0), stop=False)
                nc.tensor.matmul(
                    ps[:, :], ones_row[:, t * TOK:(t + 1) * TOK], tWb[:, :],
                    start=False, stop=True)

                # rmsnorm over q,k head slices: ms = mean(t^2) per (tok, head)
                # Square(x * 1/sqrt(DH)) accumulated over the free dim = mean(x^2)
                sq_scr = wp.tile([TOK, DH], F32, tag="sq_scr")
                ms = wp.tile([TOK, 4], F32, tag="ms")
                for g in range(4):
                    nc.scalar.activation(
                        sq_scr[:, :], ps[:, g * DH:(g + 1) * DH], AF.Square,
                        scale=float(1.0 / np.sqrt(DH)),
                        accum_out=ms[:, g:g + 1])
                nc.scalar.activation(ms[:, :], ms[:, :], AF.Sqrt, bias=EPS)
                nc.vector.reciprocal(ms[:, :], ms[:, :])
                qkn = wp.tile([TOK, 2 * HP * DH], F32R, tag="qkn")
                nc.vector.tensor_mul(
                    qkn[:, :].rearrange("p (a b) -> p a b", a=4),
                    ps[:, :VOFF].rearrange("p (a b) -> p a b", a=4),
                    ms[:, :, None].broadcast_to([TOK, 4, DH]))
                # v + ones columns straight to resident tiles
                nc.scalar.copy(vaug[t][:, :], ps[:, VOFF:WCOLS])

                # transposes: q block and k block
                psq = ptr.tile([128, TOK], F32R, tag="ptr")
                nc.tensor.transpose(psq[:, :], qkn[:, 0:128], ident[:, :])
                nc.scalar.copy(qT[:, t * TOK:(t + 1) * TOK], psq[:, :])
                psk = ptr.tile([128, TOK], F32R, tag="ptr")
                nc.tensor.transpose(psk[:, :], qkn[:, 128:256], ident[:, :])
                # fold wq*wk into kT during the PSUM->SBUF move
                nc.vector.tensor_scalar_mul(
                    kT[:, t * TOK:(t + 1) * TOK], psk[:, :], twqk[:, :])

            # ---- attention, transposed scores, max-free softmax ----
            for h in range(HP):
                hs = h * DH
                vs = h * (DH + 1)
                for qc in range(NQ):
                    qsl = slice(qc * QC, (qc + 1) * QC)
                    nkb = _kb_count(qc, mask_mode)
                    po = pout.tile([DH + 1, QC], F32, tag="po")
                    for kb in range(nkb):
                        pss = pscore.tile([TOK, QC], F32, tag="smm")
                        nc.tensor.matmul(
                            pss[:, :],
                            kT[hs:hs + DH, kb * TOK:(kb + 1) * TOK],
                            qT[hs:hs + DH, qsl],
                            start=True, stop=True)
                        pt = wp.tile([TOK, QC], F32R, tag="pt")
                        diag = mask_mode == "causal" and kb >= 4 * qc
                        if diag:
                            praw = wp.tile([TOK, QC], F32, tag="praw")
                            nc.scalar.activation(praw[:, :], pss[:, :], AF.Exp,
                                                 bias=-float(expC))
                            off = 384 - (kb * TOK - qc * QC)
                            nc.vector.tensor_mul(
                                pt[:, :], praw[:, :],
                                tstrip[:, off:off + QC])
                        elif mask_mode == "general":
                            praw = wp.tile([TOK, QC], F32, tag="praw")
                            nc.scalar.activation(praw[:, :], pss[:, :], AF.Exp,
                                                 bias=-float(expC))
                            mu8 = wp.tile([TOK, QC], U8, tag="mu8")
                            nc.sync.dma_start(
                                mu8[:, :],
                                i_maskT[kb * TOK:(kb + 1) * TOK, qsl])
                            mf = wp.tile([TOK, QC], F32, tag="mf")
                            nc.vector.tensor_scalar(
                                out=mf[:, :], in0=mu8[:, :], scalar1=-1.0,
                                scalar2=1.0, op0=ALU.mult, op1=ALU.add)
                            nc.vector.tensor_mul(pt[:, :], praw[:, :], mf[:, :])
                        else:
                            nc.scalar.activation(pt[:, :], pss[:, :], AF.Exp,
                                                 bias=-float(expC))
                        nc.tensor.matmul(
                            po[:, :], vaug[kb][:, vs:vs + DH + 1], pt[:, :],
                            start=(kb == 0), stop=(kb == nkb - 1))
                    # normalize: attnT = po[0:DH] * (1/po[DH]) broadcast
                    rec = wp.tile([1, QC], F32, tag="rec")
                    nc.vector.reciprocal(rec[:, :], po[DH:DH + 1, :])
                    bc = wp.tile([DH, QC], F32, tag="bc")
                    nc.gpsimd.partition_broadcast(bc[:, :], rec[:, :])
                    nc.vector.tensor_mul(attnT[hs:hs + DH, qsl],
                                         po[0:DH, :], bc[:, :])

            # ---- output projection ----
            for t in range(NT):
                pso = pscore.tile([TOK, D], F32, tag="smm")
                nc.tensor.matmul(pso[:, :], attnT[:, t * TOK:(t + 1) * TOK],
                                 tWo[:, :], start=True, stop=True)
                osb = wp.tile([TOK, D], F32, tag="osb")
                if has_bo:
                    nc.vector.tensor_add(osb[:, :], pso[:, :], tbo[:, :])
                else:
                    nc.scalar.copy(osb[:, :], pso[:, :])
                nc.sync.dma_start(d_part[t * TOK:(t + 1) * TOK, :], osb[:, :])

            # ---- reduce-scatter across the 4 cores of this batch ----
            nc.gpsimd.collective_compute(
                "ReduceScatter", ALU.add,
                replica_groups=[[0, 1, 2, 3], [4, 5, 6, 7]],
                ins=[d_part.ap().opt()],
                outs=[d_red.ap().opt()])
            nc.sync.dma_start(o_y[:, :], d_red[:, :])

    nc.compile()
    return nc


def _mask_mode(mask):
    if not mask.any():
        return "none"
    causal = np.triu(np.ones((S, S), dtype=bool), k=1)
    if mask.shape == (S, S) and np.array_equal(mask, causal):
        return "causal"
    return "general"


def kernel(x, mask, Wqkv, bqkv, Wo, bo, wq, wk):
    x = np.ascontiguousarray(np.asarray(x, np.float32))
    mask = np.asarray(mask, bool)
    Wqkv = np.asarray(Wqkv, np.float32)
    bqkv = np.asarray(bqkv, np.float32)
    Wo = np.asarray(Wo, np.float32)
    bo = np.asarray(bo, np.float32)
    wq = np.asarray(wq, np.float32)
    wk = np.asarray(wk, np.float32)

    mode = _mask_mode(mask)
    has_bo = bool(np.any(bo))
    wqk = wq * wk
    expC = float(max(0.0, DH * float(np.abs(wqk).max()) - 40.0))

    key = (mode, has_bo, expC)
    if key not in _cache:
        _cache[key] = _build(mode, has_bo, expC)
    nc = _cache[key]

    # strip[i, c] = 1 if c - 384 >= i  (slice at 384-d gives j - i >= d)
    strip = (np.arange(TOK + QC + 256)[None, :] - 384 >=
             np.arange(TOK)[:, None]).astype(np.float32)

    in_maps = []
    for c in range(N_CORES):
        b, hp = divmod(c, 4)
        h0 = HP * hp
        cols = []
        for part in range(3):            # q, k, v blocks
            for h in range(HP):
                w = Wqkv[:, part * D + (h0 + h) * DH: part * D + (h0 + h + 1) * DH]
                cols.append(w)
                if part == 2:
                    cols.append(np.zeros((D, 1), np.float32))
        Wp = np.concatenate(cols, axis=1)          # [D, WCOLS]
        bias = []
        for part in range(3):
            for h in range(HP):
                bias.append(bqkv[part * D + (h0 + h) * DH: part * D + (h0 + h + 1) * DH])
                if part == 2:
                    bias.append(np.ones(1, np.float32))
        Wb = np.concatenate(bias)[None, :]          # [1, WCOLS]
        WoP = np.concatenate([Wo[(h0 + h) * DH:(h0 + h + 1) * DH, :]
                              for h in range(HP)], axis=0)   # [128, D]
        m = {
            "xT": np.ascontiguousarray(x[b].T),
            "Wp": np.ascontiguousarray(Wp),
            "Wb": np.ascontiguousarray(Wb),
            "WoP": np.ascontiguousarray(WoP),
            "wqk": np.ascontiguousarray(np.tile(wqk, HP)[:, None]),
            "ident": np.eye(128, dtype=np.float32),
            "onesr": np.ones((1, S), np.float32),
        }
        if mode == "causal":
            m["strip"] = strip
        elif mode == "general":
            m["maskT"] = np.ascontiguousarray(mask.T.astype(np.uint8))
        if has_bo:
            m["boB"] = np.ascontiguousarray(np.broadcast_to(bo, (TOK, D)).copy())
        in_maps.append(m)

    res = run_bass_kernel_spmd(nc, in_maps, core_ids=list(range(N_CORES)))
    out = np.empty((B, S, D), np.float32)
    for c in range(N_CORES):
        b, hp = divmod(c, 4)
        out[b, hp * (S // 4):(hp + 1) * (S // 4), :] = res.results[c]["y"]
    return out
